# revision 1
# baseline (speedup 1.0000x reference)
"""NeRF renderer on 8 Trainium2 NeuronCores (Bass/Tile).

kernel(**inputs) takes FULL inputs (rays_o/rays_d [32768,3], MLP params,
num_steps=128) and returns the FULL [32768,9] output. Rays are sharded 8 ways
(4096 rays/core); params are replicated (baked into per-core constants).

Math: per ray, pre-activation hiddens are linear in z (H = P + z_t*Q), so the
host precomputes per-ray P/Q/Pc/Qc (and AABB near/far -> deltas). The device
evaluates relu/heads via small matmuls packed t-on-partition in PSUM, then
composites with a triangular-matmul cumsum and telescoped weights
w = (1-exp(-x)) * exp(x-S). softplus/sigmoid are built from exp+ln so the
whole kernel uses one ScalarE table set.
"""

import sys
from contextlib import ExitStack

for _p in ("/opt/trn_rl_repo", "/root/.axon_site/_ro/trn_rl_repo"):
    if _p not in sys.path:
        sys.path.insert(0, _p)

import numpy as np

N_CORES = 8
N_RAYS = 32768
R_CORE = N_RAYS // N_CORES
RC = 512
T = 128
H = 32
F32 = np.float32

Z = (np.arange(T, dtype=np.float64) / (T - 1)).astype(F32)

CONST_COLS = dict(
    h=0, sig=4096, rgb=4224, tri=4256, sum0=4384, sel=4512, red=5024,
    wd=5030, ya=5038, yb=5047, yb1=5056, dl=5065, one=5321, pq=5833,
    total=5961,
)


def _sig_rho(ul, h2, g):
    return 32 * (ul & 3) + 8 * (ul >> 2) + 4 * h2 + g


def _rgb_rho(ul, g, c2):
    return 32 * ((ul + 2) & 3) + 6 * g + c2


def build_constants(W1, b1, Wsig, Wsig_d, Wc1, bc1, Wc2, Wc2_d):
    C = {}
    lhsT_H = np.zeros((32, 64, 128), F32)
    for u in range(32):
        for g in range(4):
            for j in range(H):
                lhsT_H[u, j, 32 * g + j] = 1.0
                lhsT_H[u, H + j, 32 * g + j] = Z[4 * u + g]
    C["lhsT_H"] = lhsT_H

    Wsig2 = [np.asarray(Wsig, F32)[:, 0], np.asarray(Wsig_d, F32)[:, 0]]
    lhsT_sig = np.zeros((4, 128, 32), F32)
    for qp in range(4):
        for g in range(4):
            for h2 in range(2):
                for j in range(H):
                    lhsT_sig[qp, 32 * g + j, 8 * qp + 4 * h2 + g] = Wsig2[h2][j]
    C["lhsT_sig"] = lhsT_sig

    Wc2all = np.concatenate([np.asarray(Wc2, F32), np.asarray(Wc2_d, F32)], axis=1)
    lhsT_rgb = np.zeros((128, 32), F32)
    for g in range(4):
        for c2 in range(6):
            for j in range(H):
                lhsT_rgb[32 * g + j, 6 * g + c2] = Wc2all[j, c2]
    C["lhsT_rgb"] = lhsT_rgb

    rho_t = np.zeros(128, np.int64)
    rho_h2 = np.zeros(128, np.int64)
    for ul in range(16):
        for h2 in range(2):
            for g in range(4):
                rho = _sig_rho(ul, h2, g)
                rho_t[rho] = 4 * ul + g
                rho_h2[rho] = h2
    C["lhsT_tri"] = ((rho_h2[:, None] == rho_h2[None, :])
                     & (rho_t[:, None] <= rho_t[None, :])).astype(F32)
    C["lhsT_sum0"] = (rho_h2[:, None] == rho_h2[None, :]).astype(F32)

    lhsT_sel = np.zeros((4, 128, 128), F32)
    for beta in range(4):
        for ul in range(4 * beta, 4 * beta + 4):
            for g in range(4):
                for c2 in range(6):
                    rr = _rgb_rho(ul, g, c2)
                    h2 = 1 if c2 >= 3 else 0
                    src = np.where((rho_t == 4 * ul + g) & (rho_h2 == h2))[0]
                    lhsT_sel[beta, src[0], rr] = 1.0
    C["lhsT_sel"] = lhsT_sel

    lhsT_red = np.zeros((128, 6), F32)
    for rr in range(128):
        c24 = rr & 31
        if c24 < 24:
            lhsT_red[rr, c24 % 6] = 1.0
    C["lhsT_red"] = lhsT_red

    lhsT_wd = np.zeros((2, 128, 4), F32)
    for seg in range(2):
        for rho in range(128):
            h2 = rho_h2[rho]
            lhsT_wd[seg, rho, 2 * h2 + 0] = 1.0
            lhsT_wd[seg, rho, 2 * h2 + 1] = Z[64 * seg + rho_t[rho]]
    C["lhsT_wd"] = lhsT_wd

    # rgb is shipped as tanh(raw/2); sigmoid = 0.5*tanh + 0.5 is folded here:
    # image_final = 0.5*imgth + 0.5*ws + (1 - ws) = 0.5*imgth - 0.5*ws + 1
    lhsT_ya = np.zeros((6, 9), F32)
    for c2 in range(6):
        lhsT_ya[c2, c2 if c2 < 3 else 2 + c2] = 0.5
    C["lhsT_ya"] = lhsT_ya

    yb = np.zeros((5, 9), F32)
    yb[0, 0:3] = -0.5
    yb[0, 4] = 1.0
    yb[1, 3] = 1.0
    yb[2, 5:8] = -0.5
    yb[3, 8] = 1.0
    yb[4, 0:3] = 1.0
    yb[4, 5:8] = 1.0
    C["lhsT_yb"] = yb

    lhsT_dl = np.zeros((2, 2, 128), F32)
    for seg in range(2):
        for rho in range(128):
            tg = 64 * seg + rho_t[rho]
            lhsT_dl[seg, 0 if tg != 127 else 1, rho] = 1.0
    C["lhsT_dl"] = lhsT_dl

    # on-device P/Q/Pc/Qc build: [10, 128] from rows (A3, B3, d3, ones)
    W1 = np.asarray(W1, F32)
    b1 = np.asarray(b1, F32)
    Wc1 = np.asarray(Wc1, F32)
    bc1 = np.asarray(bc1, F32)
    pq = np.zeros((10, 128), F32)
    for j in range(H):
        for ci in range(3):
            pq[ci, j] = W1[ci, j]            # P
            pq[3 + ci, 32 + j] = W1[ci, j]   # Q
            pq[ci, 64 + j] = Wc1[ci, j]      # Pc (A part)
            pq[6 + ci, 64 + j] = Wc1[3 + ci, j]  # Pc (d part)
            pq[3 + ci, 96 + j] = Wc1[ci, j]  # Qc
        pq[9, j] = b1[j]
        pq[9, 64 + j] = bc1[j]
    C["lhsT_pq"] = pq
    return C


def pack_const_tile(C):
    CC = CONST_COLS
    ct = np.zeros((128, CC["total"]), F32)
    for u in range(32):
        ct[0:64, 128 * u:128 * u + 128] = C["lhsT_H"][u]
        ct[64:128, 128 * u:128 * u + 128] = C["lhsT_H"][u]
    for qp in range(4):
        ct[:, CC["sig"] + 32 * qp:CC["sig"] + 32 * qp + 32] = C["lhsT_sig"][qp]
    ct[:, CC["rgb"]:CC["rgb"] + 32] = C["lhsT_rgb"]
    ct[:, CC["tri"]:CC["tri"] + 128] = C["lhsT_tri"]
    ct[:, CC["sum0"]:CC["sum0"] + 128] = C["lhsT_sum0"]
    for b in range(4):
        ct[:, CC["sel"] + 128 * b:CC["sel"] + 128 * b + 128] = C["lhsT_sel"][b]
    ct[:, CC["red"]:CC["red"] + 6] = C["lhsT_red"]
    for seg in range(2):
        ct[:, CC["wd"] + 4 * seg:CC["wd"] + 4 * seg + 4] = C["lhsT_wd"][seg]
    ct[0:6, CC["ya"]:CC["ya"] + 9] = C["lhsT_ya"]
    ct[0:4, CC["yb"]:CC["yb"] + 9] = C["lhsT_yb"][0:4]
    ct[0:1, CC["yb1"]:CC["yb1"] + 9] = C["lhsT_yb"][4:5]
    for seg in range(2):
        ct[0:2, CC["dl"] + 128 * seg:CC["dl"] + 128 * seg + 128] = C["lhsT_dl"][seg]
    ct[0:1, CC["one"]:CC["one"] + 512] = 1.0
    ct[0:10, CC["pq"]:CC["pq"] + 128] = C["lhsT_pq"]
    return ct


def host_prep(rays_o, rays_d):
    """Per-ray prep -> R10 [10, N] rows (A3, B3, d3, ones), D2 [2, N]."""
    o = np.asarray(rays_o, F32)
    rd = np.asarray(rays_d, F32)
    n2 = rd[:, 0] * rd[:, 0] + rd[:, 1] * rd[:, 1] + rd[:, 2] * rd[:, 2]
    d = rd * (1.0 / np.sqrt(n2))[:, None]
    inv = 1.0 / d
    t1 = (-1.0 - o) * inv
    t2 = (1.0 - o) * inv
    near = np.maximum(np.minimum(t1, t2).max(-1), F32(0.2))
    far = np.maximum(np.maximum(t1, t2).min(-1), near + F32(1e-6))
    span = far - near
    A = o + d * near[:, None]
    B = d * span[:, None]
    N = o.shape[0]
    R12 = np.empty((12, N), F32)
    R12[0:3] = A.T
    R12[3:6] = B.T
    R12[6:9] = d.T
    R12[9] = 1.0
    R12[10] = span * (1.0 / (T - 1))
    R12[11] = span * (1.0 / T)
    return R12


def emit_nerf(tc, y_ap, x_ap, d2_ap, cst_ap, n_rays=R_CORE,
              dbg_skip_quads=False, dbg_skip_sigrgb=False,
              dbg_skip_composite=False):
    import concourse.mybir as mybir
    AF = mybir.ActivationFunctionType
    ALU = mybir.AluOpType
    f32 = mybir.dt.float32
    nc = tc.nc
    nchunk = n_rays // RC
    CC = CONST_COLS

    with ExitStack() as ctx:
        singles = ctx.enter_context(tc.tile_pool(name="singles", bufs=1))
        xpool = ctx.enter_context(tc.tile_pool(name="xpool", bufs=2))
        hpool = ctx.enter_context(tc.tile_pool(name="hpool", bufs=3))
        cpool = ctx.enter_context(tc.tile_pool(name="cpool", bufs=2))
        rgbpool = ctx.enter_context(tc.tile_pool(name="rgbpool", bufs=8))
        opool = ctx.enter_context(tc.tile_pool(name="opool", bufs=2))
        psH = ctx.enter_context(tc.tile_pool(name="psH", bufs=2, space="PSUM"))
        psHC = ctx.enter_context(tc.tile_pool(name="psHC", bufs=2, space="PSUM"))
        psSig = ctx.enter_context(tc.tile_pool(name="psSig", bufs=2, space="PSUM"))
        psRgb = ctx.enter_context(tc.tile_pool(name="psRgb", bufs=2, space="PSUM"))

        cst = singles.tile([128, CC["total"]], mybir.dt.float32r)
        nc.sync.dma_start(out=cst[:], in_=cst_ap[:])
        bf16 = mybir.dt.bfloat16
        c16 = singles.tile([128, 160], bf16)
        nc.sync.dma_start(out=c16[:], in_=d2_ap[:])

        def cs(key, off, k, w):
            c0 = CC[key] + off
            return cst[0:k, c0:c0 + w] if k != 128 else cst[:, c0:c0 + w]

        f32r = mybir.dt.float32r

        def MM(out, lhsT, rhs, **kw):
            # float32r: same bytes, 4x faster PE row rate at N>=256
            nc.tensor.matmul(out, lhsT.bitcast(f32r), rhs.bitcast(f32r), **kw)

        f16 = mybir.dt.float16
        for c in range(nchunk):
            r16 = xpool.tile([10, RC], f16, tag="r16", name=f"r16{c}")
            nc.sync.dma_start(out=r16[:], in_=x_ap[0:10, c * RC:(c + 1) * RC])
            d16 = xpool.tile([2, RC], f16, tag="d16", name=f"d16{c}")
            nc.sync.dma_start(out=d16[:], in_=x_ap[10:12, c * RC:(c + 1) * RC])
            r_c = xpool.tile([10, RC], mybir.dt.float32r, tag="rc", name=f"rc{c}")
            nc.vector.tensor_copy(r_c[:], r16[:])
            d2_c = xpool.tile([2, RC], mybir.dt.float32r, tag="d2c", name=f"d2c{c}")
            nc.vector.tensor_copy(d2_c[:], d16[:])
            x_ps = psH.tile([128, RC], f32, tag="h", name=f"xps{c}")
            MM(x_ps[:], cs("pq", 0, 10, 128), r_c[:],
                             start=True, stop=True)
            x_c = xpool.tile([128, RC], mybir.dt.float32r, tag="xc", name=f"xc{c}")
            nc.scalar.activation(x_c[:], x_ps[:], AF.Copy)

            x_sb = [None, None]
            rgb_sb = [[None] * 4, [None] * 4]
            w_sb = [None, None]
            a_sbs = []
            L_sbs = []

            for seg in range(2):
                sig_ps = psSig.tile([128, RC], f32, tag="sig", name=f"sig{c}_{seg}")
                rgb_ps = [None] * 4
                pend = None  # (ul, h_sb, hc_sb) lagged one quad for PE pipelining

                def heads(ul, h_sb, hc_sb):
                    # strip-packed dst partitions are invalid for f32r
                    # (s3d3_mm_valid_dst_partition) -> plain fp32 here
                    s = ul & 3
                    qp = ul >> 2
                    nc.tensor.matmul(
                        sig_ps[32 * s:32 * s + 32, :],
                        c16[:, 32 * qp:32 * qp + 32], h_sb[:],
                        start=(qp == 0), stop=(qp == 3),
                        tile_position=(0, 32 * s), skip_group_check=True)
                    sr = (ul + 2) & 3
                    beta = ul >> 2
                    if rgb_ps[beta] is None:
                        rgb_ps[beta] = psRgb.tile([128, RC], f32, tag="rgb",
                                                  name=f"rgbps{c}_{seg}_{beta}")
                    nc.tensor.matmul(
                        rgb_ps[beta][32 * sr:32 * sr + 32, :],
                        c16[:, 128:160], hc_sb[:],
                        start=True, stop=True,
                        tile_position=(0, 32 * sr), skip_group_check=True)

                for ul in ([] if dbg_skip_quads else range(16)):
                    u = 16 * seg + ul
                    hps = psH.tile([128, RC], f32, tag="h", name=f"h{c}_{u}")
                    MM(
                        hps[:], cst[0:64, 128 * u:128 * (u + 1)], x_c[0:64, :],
                        start=True, stop=True)
                    hcps = psHC.tile([128, RC], f32, tag="hc", name=f"hc{c}_{u}")
                    MM(
                        hcps[:], cst[64:128, 128 * u:128 * (u + 1)], x_c[64:128, :],
                        start=True, stop=True)
                    if pend is not None:
                        heads(*pend)
                    h_sb = hpool.tile([128, RC], bf16, tag="hsb", name=f"hsb{c}_{u}")
                    nc.scalar.activation(h_sb[:], hps[:], AF.Relu)
                    hc_sb = hpool.tile([128, RC], bf16, tag="hcsb", name=f"hcsb{c}_{u}")
                    nc.vector.tensor_scalar_max(hc_sb[:], hcps[:], 0.0)
                    pend = (ul, h_sb, hc_sb)
                if pend is not None:
                    heads(*pend)

                if dbg_skip_sigrgb:
                    continue
                a_sb = cpool.tile([128, RC], f32, tag="a", name=f"a{c}_{seg}")
                nc.scalar.activation(a_sb[:], sig_ps[:], AF.Exp)
                a_sbs.append(a_sb)

                # rgb evac: tanh(raw/2); the 0.5*t+0.5 sigmoid affine is
                # folded into the y-assembly constants.
                for beta in range(4):
                    r_sb = rgbpool.tile([128, RC], f32, tag="rgbsb",
                                        name=f"rgbsb{c}_{seg}_{beta}")
                    nc.scalar.activation(r_sb[:], rgb_ps[beta][:], AF.Tanh,
                                         scale=0.5)
                    rgb_sb[seg][beta] = r_sb

            if dbg_skip_sigrgb or dbg_skip_composite:
                continue
            # both Ln ops adjacent: one table-set round trip per chunk
            for seg in range(2):
                L_sb = cpool.tile([128, RC], f32, tag="L", name=f"L{c}_{seg}")
                nc.scalar.activation(L_sb[:], a_sbs[seg][:], AF.Ln, bias=1.0)
                L_sbs.append(L_sb)
            for seg in range(2):
                dl_ps = psH.tile([128, RC], f32, tag="h", name=f"dl{c}_{seg}")
                MM(
                    dl_ps[:], cs("dl", 128 * seg, 2, 128), d2_c[:],
                    start=True, stop=True)
                xs = cpool.tile([128, RC], mybir.dt.float32r, tag="x", name=f"x{c}_{seg}")
                nc.vector.tensor_tensor(xs[:], L_sbs[seg][:], dl_ps[:],
                                        op=ALU.mult)
                x_sb[seg] = xs

            for seg in range(2):
                S_ps = psHC.tile([128, RC], f32, tag="hc", name=f"S{c}_{seg}")
                MM(S_ps[:], cs("tri", 0, 128, 128), x_sb[seg][:],
                                 start=True, stop=(seg == 0))
                if seg == 1:
                    MM(S_ps[:], cs("sum0", 0, 128, 128), x_sb[0][:],
                                     start=False, stop=True)
                tmp = cpool.tile([128, RC], f32, tag="tmp", name=f"tmp{c}_{seg}")
                nc.vector.tensor_tensor(tmp[:], x_sb[seg][:], S_ps[:], op=ALU.subtract)
                E_sb = cpool.tile([128, RC], f32, tag="E", name=f"E{c}_{seg}")
                nc.scalar.activation(E_sb[:], tmp[:], AF.Exp)
                y1_sb = cpool.tile([128, RC], f32, tag="y1", name=f"y1{c}_{seg}")
                nc.scalar.activation(y1_sb[:], x_sb[seg][:], AF.Exp, scale=-1.0)
                t2 = cpool.tile([128, RC], f32, tag="t2", name=f"t2{c}_{seg}")
                nc.vector.tensor_scalar(t2[:], y1_sb[:], -1.0, 1.0,
                                        op0=ALU.mult, op1=ALU.add)
                ws_ = cpool.tile([128, RC], mybir.dt.float32r, tag="w", name=f"w{c}_{seg}")
                nc.vector.tensor_tensor(ws_[:], t2[:], E_sb[:], op=ALU.mult)
                w_sb[seg] = ws_

            img_ps = psRgb.tile([6, RC], f32, tag="rgb", name=f"img{c}")
            n_img = 0
            for seg in range(2):
                for beta in range(4):
                    wrep_ps = psSig.tile([128, RC], f32, tag="sig",
                                         name=f"wrep{c}_{seg}_{beta}")
                    MM(wrep_ps[:], cs("sel", 128 * beta, 128, 128),
                                     w_sb[seg][:], start=True, stop=True)
                    wrgb = cpool.tile([128, RC], mybir.dt.float32r, tag="wrgb",
                                      name=f"wrgb{c}_{seg}_{beta}")
                    nc.vector.tensor_tensor(wrgb[:], rgb_sb[seg][beta][:],
                                            wrep_ps[:], op=ALU.mult)
                    MM(img_ps[:], cs("red", 0, 128, 6), wrgb[:],
                                     start=(n_img == 0), stop=(n_img == 7),
                                     skip_group_check=True)
                    n_img += 1

            wd_ps = psRgb.tile([4, RC], f32, tag="rgb", name=f"wd{c}")
            MM(wd_ps[:], cs("wd", 0, 128, 4), w_sb[0][:],
                             start=True, stop=False)
            MM(wd_ps[:], cs("wd", 4, 128, 4), w_sb[1][:],
                             start=False, stop=True)

            img_sb = opool.tile([6, RC], mybir.dt.float32r, tag="img", name=f"imgsb{c}")
            nc.scalar.activation(img_sb[:], img_ps[:], AF.Copy)
            wd_sb = opool.tile([4, RC], mybir.dt.float32r, tag="wd", name=f"wdsb{c}")
            nc.scalar.activation(wd_sb[:], wd_ps[:], AF.Copy)

            y_ps = psRgb.tile([9, RC], f32, tag="rgb", name=f"y{c}")
            MM(y_ps[:], cs("ya", 0, 6, 9), img_sb[:],
                             start=True, stop=False)
            MM(y_ps[:], cs("yb", 0, 4, 9), wd_sb[:],
                             start=False, stop=False)
            MM(y_ps[:], cs("yb1", 0, 1, 9), cs("one", 0, 1, RC),
                             start=False, stop=True)
            y_sb = opool.tile([9, RC], mybir.dt.float16, tag="ysb", name=f"ysb{c}")
            nc.scalar.activation(y_sb[:], y_ps[:], AF.Copy)
            nc.sync.dma_start(out=y_ap[:, c * RC:(c + 1) * RC], in_=y_sb[:])


_CACHED = {}


def _build_runner(nc):
    """Persistent jitted SPMD runner (avoids bass2jax's per-call re-jit)."""
    import jax
    from jax.sharding import Mesh, PartitionSpec
    from jax.experimental.shard_map import shard_map
    from concourse import bass2jax

    bass2jax.install_neuronx_cc_hook()
    in_names = ["x", "cst", "cst16"]
    out_names = ["y"]
    out_avals = [jax.core.ShapedArray((9, R_CORE), np.float16)]
    all_names = in_names + out_names
    pname = nc.partition_id_tensor.name if nc.partition_id_tensor else None
    if pname is not None:
        all_names = all_names + [pname]

    def _body(*args):
        operands = list(args)
        if pname is not None:
            operands.append(bass2jax.partition_id_tensor())
        outs = bass2jax._bass_exec_p.bind(
            *operands,
            out_avals=tuple(out_avals),
            in_names=tuple(all_names),
            out_names=tuple(out_names),
            lowering_input_output_aliases=(),
            sim_require_finite=True,
            sim_require_nnan=True,
            nc=nc,
        )
        return tuple(outs)

    import jax.numpy as jnp
    from jax.sharding import NamedSharding

    devices = jax.devices()[:N_CORES]
    mesh = Mesh(np.asarray(devices), ("core",))
    sh = NamedSharding(mesh, PartitionSpec("core"))

    sharded = jax.jit(
        shard_map(_body, mesh=mesh,
                  in_specs=(PartitionSpec("core"),) * 4,
                  out_specs=(PartitionSpec("core"),) * len(out_names),
                  check_rep=False),
        donate_argnums=(3,), keep_unused=True)
    zfn = jax.jit(lambda: jnp.zeros((N_CORES * 9, R_CORE), np.float16),
                  out_shardings=sh)
    zpool = [zfn() for _ in range(4)]
    cst_cache = {}

    def run(x_cat, cst, cst16, cst_key):
        if cst_key not in cst_cache:
            cst_cache.clear()
            cst_cache[cst_key] = (
                jax.device_put(np.concatenate([cst] * N_CORES, axis=0), sh),
                jax.device_put(np.concatenate([cst16] * N_CORES, axis=0), sh))
        z = zpool.pop() if zpool else zfn()
        cd, cd16 = cst_cache[cst_key]
        (y_out,) = sharded(x_cat, cd, cd16, z)
        out = np.asarray(y_out).reshape(N_CORES, 9, R_CORE)
        if len(zpool) < 2:
            zpool.append(zfn())
        return out

    return run


def _build_module(**dbg):
    import concourse.bacc as bacc
    import concourse.tile as tile
    import concourse.mybir as mybir

    nc = bacc.Bacc("TRN2", target_bir_lowering=False, debug=False)
    x = nc.dram_tensor("x", [12, R_CORE], mybir.dt.float16, kind="ExternalInput")
    cst = nc.dram_tensor("cst", [128, CONST_COLS["total"]], mybir.dt.float32r,
                         kind="ExternalInput")
    cst16 = nc.dram_tensor("cst16", [128, 160], mybir.dt.bfloat16,
                           kind="ExternalInput")
    y = nc.dram_tensor("y", [9, R_CORE], mybir.dt.float16, kind="ExternalOutput")
    with tile.TileContext(nc) as tc:
        emit_nerf(tc, y.ap(), x.ap(), cst16.ap(), cst.ap(), n_rays=R_CORE, **dbg)
    nc.compile()
    return nc


def kernel(rays_o, rays_d, W1, b1, Wsig, Wsig_d, Wc1, bc1, Wc2, Wc2_d, num_steps):
    import hashlib

    assert int(num_steps) == T
    weights = [np.ascontiguousarray(np.asarray(a, F32))
               for a in (W1, b1, Wsig, Wsig_d, Wc1, bc1, Wc2, Wc2_d)]
    key = hashlib.md5(b"".join(a.tobytes() for a in weights)).hexdigest()

    if _CACHED.get("cst_key") != key:
        C = build_constants(*weights)
        _CACHED["cst"] = pack_const_tile(C)
        c16 = np.zeros((128, 160), np.float32)
        for qp in range(4):
            c16[:, 32 * qp:32 * qp + 32] = C["lhsT_sig"][qp]
        c16[:, 128:160] = C["lhsT_rgb"]
        _CACHED["cst16"] = c16.astype(np.dtype("bfloat16")) if hasattr(np, "bfloat16") else None
        if _CACHED["cst16"] is None:
            import ml_dtypes
            _CACHED["cst16"] = c16.astype(ml_dtypes.bfloat16)
        _CACHED["cst_key"] = key
    cst = _CACHED["cst"]

    R12 = host_prep(rays_o, rays_d).astype(np.float16)
    # concat over cores: [N_CORES*12, R_CORE] (shard_map splits on axis 0)
    x_cat = np.ascontiguousarray(
        R12.reshape(12, N_CORES, R_CORE).transpose(1, 0, 2).reshape(
            N_CORES * 12, R_CORE))

    if "run" not in _CACHED:
        _CACHED["nc"] = _build_module()
        _CACHED["run"] = _build_runner(_CACHED["nc"])

    y = _CACHED["run"](x_cat, cst, _CACHED["cst16"], key)  # [N_CORES, 9, R_CORE] fp16
    out = np.concatenate([y[cidx].T for cidx in range(N_CORES)], axis=0)
    return np.ascontiguousarray(out.astype(np.float32))


if __name__ == "__main__":
    rng = np.random.default_rng(0)
    ins = {
        "rays_o": (rng.random((N_RAYS, 3), dtype=np.float32) - 0.5),
        "rays_d": rng.standard_normal((N_RAYS, 3)).astype(np.float32),
        "W1": rng.standard_normal((3, 32)).astype(np.float32) * 0.5,
        "b1": np.zeros((32,), np.float32),
        "Wsig": rng.standard_normal((32, 1)).astype(np.float32) * 0.5,
        "Wsig_d": rng.standard_normal((32, 1)).astype(np.float32) * 0.5,
        "Wc1": rng.standard_normal((6, 32)).astype(np.float32) * 0.5,
        "bc1": np.zeros((32,), np.float32),
        "Wc2": rng.standard_normal((32, 3)).astype(np.float32) * 0.5,
        "Wc2_d": rng.standard_normal((32, 3)).astype(np.float32) * 0.5,
        "num_steps": 128,
    }
    out = kernel(**ins)
    print("out", out.shape, out.dtype, np.isfinite(out).all())



# revision 19
# speedup vs baseline: 209.6743x; 209.6743x over previous
"""NeRF renderer on 8 Trainium2 NeuronCores (Bass/Tile).

kernel(**inputs) takes FULL inputs (rays_o/rays_d [32768,3], MLP params,
num_steps=128) and returns the FULL [32768,9] output. Rays are sharded 8 ways
(4096 rays/core); params are replicated (baked into per-core constants).

Math: per ray, pre-activation hiddens are linear in z (H = P + z_t*Q), so the
host precomputes per-ray P/Q/Pc/Qc (and AABB near/far -> deltas). The device
evaluates relu/heads via small matmuls packed t-on-partition in PSUM, then
composites with a triangular-matmul cumsum and telescoped weights
w = exp(x-S) - exp(-S) (S = inclusive cumsum of x = delta*sigma).

v2 layout: h and hc for one t-quad share a [128,1024] PSUM tile (2 banks) so a
single relu evacuation (alternating ScalarE/VectorE) serves both trunks; rgb
PSUM is evacuated per-beta to keep the bank ring at 2; pq/dl matmuls take the
DMA'd fp16 rays directly (no cast ops); the y assembly is one matmul with the
background-color term folded into a per-partition bias on the output copy.
"""

import sys
from contextlib import ExitStack

for _p in ("/opt/trn_rl_repo", "/root/.axon_site/_ro/trn_rl_repo"):
    if _p not in sys.path:
        sys.path.insert(0, _p)

import numpy as np

N_CORES = 8
N_RAYS = 32768
R_CORE = N_RAYS // N_CORES
RC = 512
T = 128
H = 32
F32 = np.float32

Z = (np.arange(T, dtype=np.float64) / (T - 1)).astype(F32)

# f32r constant tile columns
CONST_COLS = dict(
    h=0, tri=4096, sum0=4224, sel=4352, red=4864, wd=4873, ybias=4891,
    total=4892,
)
# fp16 constant tile columns ([10, *]): pq lhsT + dl lhsT per seg
CXTRA_COLS = dict(pq=0, dl=128, total=384)


def _sig_rho(ul, h2, g):
    return 32 * (ul & 3) + 8 * (ul >> 2) + 4 * h2 + g


def _rgb_rho(ul, g, c2):
    return 32 * ((ul + 2) & 3) + 6 * g + c2


def build_constants(W1, b1, Wsig, Wsig_d, Wc1, bc1, Wc2, Wc2_d):
    C = {}
    lhsT_H = np.zeros((32, 64, 128), F32)
    for u in range(32):
        for g in range(4):
            for j in range(H):
                lhsT_H[u, j, 32 * g + j] = 1.0
                lhsT_H[u, H + j, 32 * g + j] = Z[4 * u + g]
    C["lhsT_H"] = lhsT_H

    Wsig2 = [np.asarray(Wsig, F32)[:, 0], np.asarray(Wsig_d, F32)[:, 0]]
    lhsT_sig = np.zeros((4, 128, 32), F32)
    for qp in range(4):
        for g in range(4):
            for h2 in range(2):
                for j in range(H):
                    lhsT_sig[qp, 32 * g + j, 8 * qp + 4 * h2 + g] = Wsig2[h2][j]
    C["lhsT_sig"] = lhsT_sig

    Wc2all = np.concatenate([np.asarray(Wc2, F32), np.asarray(Wc2_d, F32)], axis=1)
    lhsT_rgb = np.zeros((128, 32), F32)
    for g in range(4):
        for c2 in range(6):
            for j in range(H):
                lhsT_rgb[32 * g + j, 6 * g + c2] = Wc2all[j, c2]
    C["lhsT_rgb"] = lhsT_rgb

    rho_t = np.zeros(128, np.int64)
    rho_h2 = np.zeros(128, np.int64)
    for ul in range(16):
        for h2 in range(2):
            for g in range(4):
                rho = _sig_rho(ul, h2, g)
                rho_t[rho] = 4 * ul + g
                rho_h2[rho] = h2
    C["lhsT_tri"] = ((rho_h2[:, None] == rho_h2[None, :])
                     & (rho_t[:, None] <= rho_t[None, :])).astype(F32)
    C["lhsT_sum0"] = (rho_h2[:, None] == rho_h2[None, :]).astype(F32)

    lhsT_sel = np.zeros((4, 128, 128), F32)
    for beta in range(4):
        for ul in range(4 * beta, 4 * beta + 4):
            for g in range(4):
                for c2 in range(6):
                    rr = _rgb_rho(ul, g, c2)
                    h2 = 1 if c2 >= 3 else 0
                    src = np.where((rho_t == 4 * ul + g) & (rho_h2 == h2))[0]
                    lhsT_sel[beta, src[0], rr] = 1.0
    C["lhsT_sel"] = lhsT_sel

    lhsT_red = np.zeros((128, 6), F32)
    for rr in range(128):
        c24 = rr & 31
        if c24 < 24:
            lhsT_red[rr, c24 % 6] = 1.0
    C["lhsT_red"] = lhsT_red

    lhsT_wd = np.zeros((2, 128, 4), F32)
    for seg in range(2):
        for rho in range(128):
            h2 = rho_h2[rho]
            lhsT_wd[seg, rho, 2 * h2 + 0] = 1.0
            lhsT_wd[seg, rho, 2 * h2 + 1] = Z[64 * seg + rho_t[rho]]
    C["lhsT_wd"] = lhsT_wd

    # y assembly folded into the reductions.  rgb is shipped as tanh(raw/2);
    # sigmoid = 0.5*tanh + 0.5 folded here:
    # image_final = 0.5*imgth + 0.5*ws + (1 - ws) = 0.5*imgth - 0.5*ws + 1
    # with the "+1" applied as a per-partition bias on the output copy.
    # y rows: image(3) | depth(1) | ws(1) | image_d(3) | depth_d(1)
    ylhs_img = np.zeros((6, 9), F32)
    for c2 in range(6):
        ylhs_img[c2, c2 if c2 < 3 else 2 + c2] = 0.5
    ylhs_wd = np.zeros((4, 9), F32)
    ylhs_wd[0, 0:3] = -0.5
    ylhs_wd[0, 4] = 1.0
    ylhs_wd[1, 3] = 1.0
    ylhs_wd[2, 5:8] = -0.5
    ylhs_wd[3, 8] = 1.0
    C["lhsT_red_y"] = C["lhsT_red"] @ ylhs_img          # [128, 9]
    C["lhsT_wd_y"] = np.stack([C["lhsT_wd"][s_] @ ylhs_wd for s_ in range(2)])
    C["ybias"] = np.array([1, 1, 1, 0, 0, 1, 1, 1, 0], F32)

    lhsT_dl = np.zeros((2, 2, 128), F32)
    for seg in range(2):
        for rho in range(128):
            tg = 64 * seg + rho_t[rho]
            lhsT_dl[seg, 0 if tg != 127 else 1, rho] = 1.0
    C["lhsT_dl"] = lhsT_dl

    # pq lhsT [10, 128] applied directly to the fp16 ray rows
    # (A3, B3, d3, ones): columns 0-31 P, 32-63 Q, 64-95 Pc, 96-127 Qc
    W1 = np.asarray(W1, F32)
    b1 = np.asarray(b1, F32)
    Wc1 = np.asarray(Wc1, F32)
    bc1 = np.asarray(bc1, F32)
    pq = np.zeros((10, 128), F32)
    for j in range(H):
        for ci in range(3):
            pq[ci, j] = W1[ci, j]            # P
            pq[3 + ci, 32 + j] = W1[ci, j]   # Q
            pq[ci, 64 + j] = Wc1[ci, j]      # Pc (A part)
            pq[6 + ci, 64 + j] = Wc1[3 + ci, j]  # Pc (d part)
            pq[3 + ci, 96 + j] = Wc1[ci, j]  # Qc
        pq[9, j] = b1[j]
        pq[9, 64 + j] = bc1[j]
    C["lhsT_pq"] = pq
    return C


def pack_const_tiles(C):
    CC = CONST_COLS
    ct = np.zeros((128, CC["total"]), F32)
    for u in range(32):
        ct[0:64, 128 * u:128 * u + 128] = C["lhsT_H"][u]
        ct[64:128, 128 * u:128 * u + 128] = C["lhsT_H"][u]
    ct[:, CC["tri"]:CC["tri"] + 128] = C["lhsT_tri"]
    ct[:, CC["sum0"]:CC["sum0"] + 128] = C["lhsT_sum0"]
    for b in range(4):
        ct[:, CC["sel"] + 128 * b:CC["sel"] + 128 * b + 128] = C["lhsT_sel"][b]
    ct[:, CC["red"]:CC["red"] + 9] = C["lhsT_red_y"]
    for seg in range(2):
        ct[:, CC["wd"] + 9 * seg:CC["wd"] + 9 * seg + 9] = C["lhsT_wd_y"][seg]
    ct[0:9, CC["ybias"]] = C["ybias"]

    CX = CXTRA_COLS
    cx = np.zeros((10, CX["total"]), F32)
    cx[0:10, CX["pq"]:CX["pq"] + 128] = C["lhsT_pq"]
    for seg in range(2):
        cx[0:2, CX["dl"] + 128 * seg:CX["dl"] + 128 * seg + 128] = C["lhsT_dl"][seg]

    c16 = np.zeros((128, 160), F32)
    for qp in range(4):
        c16[:, 32 * qp:32 * qp + 32] = C["lhsT_sig"][qp]
    c16[:, 128:160] = C["lhsT_rgb"]
    return ct, cx.astype(np.float16), c16


def host_prep(rays_o, rays_d):
    """Per-ray prep -> [12, N] rows (A3, B3, d3, ones, dl0, dl1)."""
    o = np.asarray(rays_o, F32)
    rd = np.asarray(rays_d, F32)
    n2 = rd[:, 0] * rd[:, 0] + rd[:, 1] * rd[:, 1] + rd[:, 2] * rd[:, 2]
    d = rd * (1.0 / np.sqrt(n2))[:, None]
    inv = 1.0 / d
    t1 = (-1.0 - o) * inv
    t2 = (1.0 - o) * inv
    near = np.maximum(np.minimum(t1, t2).max(-1), F32(0.2))
    far = np.maximum(np.maximum(t1, t2).min(-1), near + F32(1e-6))
    span = far - near
    A = o + d * near[:, None]
    B = d * span[:, None]
    N = o.shape[0]
    R12 = np.empty((12, N), F32)
    R12[0:3] = A.T
    R12[3:6] = B.T
    R12[6:9] = d.T
    R12[9] = 1.0
    R12[10] = span * (1.0 / (T - 1))
    R12[11] = span * (1.0 / T)
    return R12


def emit_nerf(tc, y_ap, x_ap, cst_ap, cx_ap, c16_ap, n_rays=R_CORE):
    import concourse.mybir as mybir
    AF = mybir.ActivationFunctionType
    ALU = mybir.AluOpType
    f32 = mybir.dt.float32
    f32r = mybir.dt.float32r
    bf16 = mybir.dt.bfloat16
    f16 = mybir.dt.float16
    nc = tc.nc
    nchunk = n_rays // RC
    CC = CONST_COLS
    CX = CXTRA_COLS

    with ExitStack() as ctx:
        singles = ctx.enter_context(tc.tile_pool(name="singles", bufs=1))
        xpool = ctx.enter_context(tc.tile_pool(name="xpool", bufs=2))
        hpool = ctx.enter_context(tc.tile_pool(name="hpool", bufs=38))
        cpool = ctx.enter_context(tc.tile_pool(name="cpool", bufs=2))
        rgbpool = ctx.enter_context(tc.tile_pool(name="rgbpool", bufs=8))
        opool = ctx.enter_context(tc.tile_pool(name="opool", bufs=2))
        psBig = ctx.enter_context(tc.tile_pool(name="psBig", bufs=2, space="PSUM"))
        psSig = ctx.enter_context(tc.tile_pool(name="psSig", bufs=2, space="PSUM"))
        psAux = ctx.enter_context(tc.tile_pool(name="psAux", bufs=2, space="PSUM"))

        cx16 = singles.tile([10, CX["total"]], f16)
        nc.sync.dma_start(out=cx16[:], in_=cx_ap[:])
        c16 = singles.tile([128, 160], bf16)
        nc.sync.dma_start(out=c16[:], in_=c16_ap[:])
        cst = singles.tile([128, CC["total"]], f32r)
        nc.sync.dma_start(out=cst[:], in_=cst_ap[:])

        def cs(key, off, k, w):
            c0 = CC[key] + off
            return cst[0:k, c0:c0 + w] if k != 128 else cst[:, c0:c0 + w]

        def MM(out, lhsT, rhs, **kw):
            # float32r: same bytes, 4x faster PE row rate at N>=256
            nc.tensor.matmul(out, lhsT.bitcast(f32r), rhs.bitcast(f32r), **kw)

        def phase_b(c, st):
            """Hidden evals + relu evacs + sigma head matmuls.

            Generator: yields once per t-quad (32 times) so the driver can
            interleave the previous chunk's composite between quads.
            """
            r16 = xpool.tile([10, RC], f16, tag="r16", name=f"r16{c}")
            nc.sync.dma_start(out=r16[:], in_=x_ap[0:10, c * RC:(c + 1) * RC])
            d16 = xpool.tile([2, RC], f16, tag="d16", name=f"d16{c}")
            nc.sync.dma_start(out=d16[:], in_=x_ap[10:12, c * RC:(c + 1) * RC])
            st["d16"] = d16
            x_ps = psSig.tile([128, RC], f32, tag="sig", name=f"xps{c}")
            nc.tensor.matmul(x_ps[:], cx16[0:10, CX["pq"]:CX["pq"] + 128],
                             r16[:], start=True, stop=True)
            x_c = xpool.tile([128, RC], f32r, tag="xc", name=f"xc{c}")
            nc.scalar.activation(x_c[:], x_ps[:], AF.Copy)

            sig_ps = [None, None]
            st["a"] = [None, None]
            st["husbs"] = husbs = []

            def sig_head(u, husb):
                ul = u & 15
                s = ul & 3
                qp = ul >> 2
                seg = u >> 4
                nc.tensor.matmul(
                    sig_ps[seg][32 * s:32 * s + 32, :],
                    c16[:, 32 * qp:32 * qp + 32], husb[:, 0:RC],
                    start=(qp == 0), stop=(qp == 3),
                    tile_position=(0, 32 * s), skip_group_check=True)

            pend = []  # (u, husb) lagged 3 quads so evacs stay ahead of PE
            for u in range(32):
                seg = u >> 4
                if (u & 15) == 0:
                    sig_ps[seg] = psSig.tile([128, RC], f32, tag="sig",
                                             name=f"sig{c}_{seg}")
                hbig = psBig.tile([128, 2 * RC], f32, tag="big", name=f"hb{c}_{u}")
                MM(hbig[:, 0:RC], cst[0:64, 128 * u:128 * (u + 1)], x_c[0:64, :],
                   start=True, stop=True)
                MM(hbig[:, RC:2 * RC], cst[64:128, 128 * u:128 * (u + 1)],
                   x_c[64:128, :], start=True, stop=True)
                if len(pend) >= 3:
                    sig_head(*pend.pop(0))
                husb = hpool.tile([128, 2 * RC], bf16, tag="husb", bufs=38,
                                  name=f"hu{c}_{u}")
                # Bresenham split of the 32 relu evacs: ~15 on ScalarE,
                # ~17 on VectorE (balances total per-engine busy time)
                if (u * 18) // 32 != ((u + 1) * 18) // 32:
                    nc.scalar.activation(husb[:], hbig[:], AF.Relu)
                else:
                    nc.vector.tensor_scalar_max(husb[:], hbig[:], 0.0)
                husbs.append(husb)
                pend.append((u, husb))
                if (u & 15) == 15:
                    for p in pend:
                        sig_head(*p)
                    pend = []
                    # sigma pre-activations complete for this seg
                    a_sb = cpool.tile([128, RC], f32, tag="a", name=f"a{c}_{seg}")
                    nc.scalar.activation(a_sb[:], sig_ps[seg][:], AF.Exp)
                    st["a"][seg] = a_sb
                yield

        def composite(c, st):
            """B2 (rgb heads + tanh) then C1 (softplus/cumsum/weights) then
            C2 (replication + weighted reduction).  Generator yielding at op
            boundaries; interleaved into the NEXT chunk's phase B."""
            d16 = st["d16"]
            husbs = st["husbs"]
            a_sbs = st["a"]

            # B2: rgb head matmuls + tanh evacs (frees husb slots early)
            rgb_sb = [[None] * 4, [None] * 4]
            for seg in range(2):
                for beta in range(4):
                    rps = psAux.tile([128, RC], f32, tag="aux",
                                     name=f"rgbps{c}_{seg}_{beta}")
                    for ul in range(4 * beta, 4 * beta + 4):
                        sr = (ul + 2) & 3
                        nc.tensor.matmul(
                            rps[32 * sr:32 * sr + 32, :],
                            c16[:, 128:160], husbs[16 * seg + ul][:, RC:2 * RC],
                            start=True, stop=True,
                            tile_position=(0, 32 * sr), skip_group_check=True)
                        if ul & 1:
                            yield
                    r_sb = rgbpool.tile([128, RC], f32, tag="rgbsb",
                                        name=f"rgbsb{c}_{seg}_{beta}")
                    nc.scalar.activation(r_sb[:], rps[:], AF.Tanh, scale=0.5)
                    rgb_sb[seg][beta] = r_sb
            st["husbs"] = []
            # tiny column derived from the last tanh output: used as the Ln
            # bias (=1.0) so Ln cannot be scheduled before the tanh group
            # (keeps one table-set switch per group per chunk)
            dep = cpool.tile([128, 1], f32, tag="dep", name=f"dep{c}")
            nc.vector.tensor_scalar(dep[:], rgb_sb[1][3][:, 0:1], 0.0, 1.0,
                                    op0=ALU.mult, op1=ALU.add)
            yield

            # C1: softplus (exp+ln), deltas, cumsum, weights
            dl_ps = [None, None]
            for seg in range(2):
                dl = psAux.tile([128, RC], f32, tag="aux", name=f"dl{c}_{seg}")
                nc.tensor.matmul(dl[:], cx16[0:2, CX["dl"] + 128 * seg:
                                              CX["dl"] + 128 * (seg + 1)],
                                 d16[:], start=True, stop=True)
                dl_ps[seg] = dl
            yield
            L_sbs = []
            for seg in range(2):
                L_sb = cpool.tile([128, RC], f32, tag="L", name=f"L{c}_{seg}")
                nc.scalar.activation(L_sb[:], a_sbs[seg][:], AF.Ln, bias=dep[:])
                L_sbs.append(L_sb)
            yield
            x_sb = [None, None]
            for seg in range(2):
                xs = cpool.tile([128, RC], f32r, tag="x", name=f"x{c}_{seg}")
                nc.vector.tensor_tensor(xs[:], L_sbs[seg][:], dl_ps[seg][:],
                                        op=ALU.mult)
                x_sb[seg] = xs
            yield
            w_sb = [None, None]
            for seg in range(2):
                S_ps = psAux.tile([128, RC], f32, tag="aux", name=f"S{c}_{seg}")
                MM(S_ps[:], cs("tri", 0, 128, 128), x_sb[seg][:],
                   start=True, stop=(seg == 0))
                if seg == 1:
                    MM(S_ps[:], cs("sum0", 0, 128, 128), x_sb[0][:],
                       start=False, stop=True)
                tmp = cpool.tile([128, RC], f32, tag="tmp", name=f"tmp{c}_{seg}")
                nc.vector.tensor_tensor(tmp[:], x_sb[seg][:], S_ps[:],
                                        op=ALU.subtract)
                E_sb = cpool.tile([128, RC], f32r, tag="E", name=f"E{c}_{seg}")
                nc.scalar.activation(E_sb[:], tmp[:], AF.Exp)
                F_sb = cpool.tile([128, RC], f32r, tag="F", name=f"F{c}_{seg}")
                nc.scalar.activation(F_sb[:], S_ps[:], AF.Exp, scale=-1.0)
                ws_ = cpool.tile([128, RC], f32r, tag="w", name=f"w{c}_{seg}")
                nc.vector.tensor_tensor(ws_[:], E_sb[:], F_sb[:],
                                        op=ALU.subtract)
                w_sb[seg] = ws_
                yield

            # C2: weight replication, weighted reduction into y
            y_ps = psAux.tile([9, RC], f32, tag="aux", name=f"y{c}")
            MM(y_ps[:], cs("wd", 0, 128, 9), w_sb[0][:],
               start=True, stop=False, skip_group_check=True)
            MM(y_ps[:], cs("wd", 9, 128, 9), w_sb[1][:],
               start=False, stop=False, skip_group_check=True)
            yield
            n_img = 0
            for seg in range(2):
                for beta in range(4):
                    wrep_ps = psAux.tile([128, RC], f32, tag="aux",
                                         name=f"wrep{c}_{seg}_{beta}")
                    MM(wrep_ps[:], cs("sel", 128 * beta, 128, 128),
                       w_sb[seg][:], start=True, stop=True)
                    wrgb = cpool.tile([128, RC], f32r, tag="wrgb",
                                      name=f"wrgb{c}_{seg}_{beta}")
                    nc.vector.tensor_tensor(wrgb[:], rgb_sb[seg][beta][:],
                                            wrep_ps[:], op=ALU.mult)
                    MM(y_ps[:], cs("red", 0, 128, 9), wrgb[:],
                       start=False, stop=(n_img == 7),
                       skip_group_check=True)
                    n_img += 1
                    yield

            y_sb = opool.tile([9, RC], f16, tag="ysb", name=f"ysb{c}")
            nc.scalar.activation(y_sb[:], y_ps[:], AF.Identity,
                                 bias=cs("ybias", 0, 9, 1))
            nc.sync.dma_start(out=y_ap[:, c * RC:(c + 1) * RC], in_=y_sb[:])

        # software pipeline: interleave composite(c-1) into phase_b(c)
        states = {}
        prev_comp = None
        for c in range(nchunk):
            states[c] = {}
            bgen = phase_b(c, states[c])
            for _ in bgen:
                if prev_comp is not None:
                    next(prev_comp, None)
            if prev_comp is not None:
                for _ in prev_comp:
                    pass
                del states[c - 1]
            prev_comp = composite(c, states[c])
        for _ in prev_comp:
            pass


_CACHED = {}


def _build_runner(nc):
    """Persistent jitted SPMD runner (avoids bass2jax's per-call re-jit)."""
    import jax
    from jax.sharding import Mesh, PartitionSpec
    from jax.experimental.shard_map import shard_map
    from concourse import bass2jax

    bass2jax.install_neuronx_cc_hook()
    in_names = ["x", "cst", "cxtra", "cst16"]
    out_names = ["y"]
    out_avals = [jax.core.ShapedArray((9, R_CORE), np.float16)]
    all_names = in_names + out_names
    pname = nc.partition_id_tensor.name if nc.partition_id_tensor else None
    if pname is not None:
        all_names = all_names + [pname]

    def _body(*args):
        operands = list(args)
        if pname is not None:
            operands.append(bass2jax.partition_id_tensor())
        outs = bass2jax._bass_exec_p.bind(
            *operands,
            out_avals=tuple(out_avals),
            in_names=tuple(all_names),
            out_names=tuple(out_names),
            lowering_input_output_aliases=(),
            sim_require_finite=True,
            sim_require_nnan=True,
            nc=nc,
        )
        return tuple(outs)

    import jax.numpy as jnp
    from jax.sharding import NamedSharding

    devices = jax.devices()[:N_CORES]
    mesh = Mesh(np.asarray(devices), ("core",))
    sh = NamedSharding(mesh, PartitionSpec("core"))

    sharded = jax.jit(
        shard_map(_body, mesh=mesh,
                  in_specs=(PartitionSpec("core"),) * 5,
                  out_specs=(PartitionSpec("core"),) * len(out_names),
                  check_rep=False),
        keep_unused=True)
    zfn = jax.jit(lambda: jnp.zeros((N_CORES * 9, R_CORE), np.float16),
                  out_shardings=sh)
    cst_cache = {}

    def run(x_cat, cst, cx, c16, cst_key):
        if cst_key not in cst_cache:
            cst_cache.clear()
            cst_cache[cst_key] = (
                jax.device_put(np.concatenate([cst] * N_CORES, axis=0), sh),
                jax.device_put(np.concatenate([cx] * N_CORES, axis=0), sh),
                jax.device_put(np.concatenate([c16] * N_CORES, axis=0), sh))
        # z is created fresh per call: besides providing the output-alias
        # operand, executing the (different) zeros NEFF between kernel
        # executions resets device state -- back-to-back executions of the
        # same NEFF otherwise see stale state and produce wrong results.
        z = zfn()
        cd, cxd, c16d = cst_cache[cst_key]
        (y_out,) = sharded(x_cat, cd, cxd, c16d, z)
        out = np.asarray(y_out).reshape(N_CORES, 9, R_CORE)
        return out

    return run


def _build_module():
    import concourse.bacc as bacc
    import concourse.tile as tile
    import concourse.mybir as mybir

    nc = bacc.Bacc("TRN2", target_bir_lowering=False, debug=False)
    x = nc.dram_tensor("x", [12, R_CORE], mybir.dt.float16, kind="ExternalInput")
    cst = nc.dram_tensor("cst", [128, CONST_COLS["total"]], mybir.dt.float32r,
                         kind="ExternalInput")
    cx = nc.dram_tensor("cxtra", [10, CXTRA_COLS["total"]], mybir.dt.float16,
                        kind="ExternalInput")
    c16 = nc.dram_tensor("cst16", [128, 160], mybir.dt.bfloat16,
                         kind="ExternalInput")
    y = nc.dram_tensor("y", [9, R_CORE], mybir.dt.float16, kind="ExternalOutput")
    with tile.TileContext(nc) as tc:
        emit_nerf(tc, y.ap(), x.ap(), cst.ap(), cx.ap(), c16.ap(), n_rays=R_CORE)
    nc.compile()
    return nc


def _prep_consts(weights):
    import hashlib
    key = hashlib.md5(b"".join(a.tobytes() for a in weights)).hexdigest()
    if _CACHED.get("cst_key") != key:
        C = build_constants(*weights)
        ct, cx, c16 = pack_const_tiles(C)
        _CACHED["cst"] = ct
        _CACHED["cxtra"] = cx
        try:
            import ml_dtypes
            _CACHED["cst16"] = c16.astype(ml_dtypes.bfloat16)
        except ImportError:
            _CACHED["cst16"] = c16.astype(np.dtype("bfloat16"))
        _CACHED["cst_key"] = key
    return key


def kernel(rays_o, rays_d, W1, b1, Wsig, Wsig_d, Wc1, bc1, Wc2, Wc2_d, num_steps):
    assert int(num_steps) == T
    weights = [np.ascontiguousarray(np.asarray(a, F32))
               for a in (W1, b1, Wsig, Wsig_d, Wc1, bc1, Wc2, Wc2_d)]
    key = _prep_consts(weights)

    R12 = host_prep(rays_o, rays_d).astype(np.float16)
    # concat over cores: [N_CORES*12, R_CORE] (shard_map splits on axis 0)
    x_cat = np.ascontiguousarray(
        R12.reshape(12, N_CORES, R_CORE).transpose(1, 0, 2).reshape(
            N_CORES * 12, R_CORE))

    if "run" not in _CACHED:
        _CACHED["nc"] = _build_module()
        _CACHED["run"] = _build_runner(_CACHED["nc"])

    y = _CACHED["run"](x_cat, _CACHED["cst"], _CACHED["cxtra"],
                       _CACHED["cst16"], key)  # [N_CORES, 9, R_CORE] fp16
    out = np.concatenate([y[cidx].T for cidx in range(N_CORES)], axis=0)
    return np.ascontiguousarray(out.astype(np.float32))


if __name__ == "__main__":
    rng = np.random.default_rng(0)
    ins = {
        "rays_o": (rng.random((N_RAYS, 3), dtype=np.float32) - 0.5),
        "rays_d": rng.standard_normal((N_RAYS, 3)).astype(np.float32),
        "W1": rng.standard_normal((3, 32)).astype(np.float32) * 0.5,
        "b1": np.zeros((32,), np.float32),
        "Wsig": rng.standard_normal((32, 1)).astype(np.float32) * 0.5,
        "Wsig_d": rng.standard_normal((32, 1)).astype(np.float32) * 0.5,
        "Wc1": rng.standard_normal((6, 32)).astype(np.float32) * 0.5,
        "bc1": np.zeros((32,), np.float32),
        "Wc2": rng.standard_normal((32, 3)).astype(np.float32) * 0.5,
        "Wc2_d": rng.standard_normal((32, 3)).astype(np.float32) * 0.5,
        "num_steps": 128,
    }
    out = kernel(**ins)
    print("out", out.shape, out.dtype, np.isfinite(out).all())


# revision 20
# speedup vs baseline: 222.8353x; 1.0628x over previous
"""NeRF renderer on 8 Trainium2 NeuronCores (Bass/Tile).

kernel(**inputs) takes FULL inputs (rays_o/rays_d [32768,3], MLP params,
num_steps=128) and returns the FULL [32768,9] output. Rays are sharded 8 ways
(4096 rays/core); params are replicated (baked into per-core constants).

Math: per ray, pre-activation hiddens are linear in z (H = P + z_t*Q), so the
host precomputes per-ray P/Q/Pc/Qc (and AABB near/far -> deltas). The device
evaluates relu/heads via small matmuls packed t-on-partition in PSUM, then
composites with a triangular-matmul cumsum and telescoped weights
w = exp(x-S) - exp(-S) (S = inclusive cumsum of x = delta*sigma).

v2 layout: h and hc for one t-quad share a [128,1024] PSUM tile (2 banks) so a
single relu evacuation (alternating ScalarE/VectorE) serves both trunks; rgb
PSUM is evacuated per-beta to keep the bank ring at 2; pq/dl matmuls take the
DMA'd fp16 rays directly (no cast ops); the y assembly is one matmul with the
background-color term folded into a per-partition bias on the output copy.
"""

import sys
from contextlib import ExitStack

for _p in ("/opt/trn_rl_repo", "/root/.axon_site/_ro/trn_rl_repo"):
    if _p not in sys.path:
        sys.path.insert(0, _p)

import numpy as np

N_CORES = 8
N_RAYS = 32768
R_CORE = N_RAYS // N_CORES
RC = 512
T = 128
H = 32
F32 = np.float32

Z = (np.arange(T, dtype=np.float64) / (T - 1)).astype(F32)

# f32r constant tile columns
CONST_COLS = dict(
    h=0, tri=4096, sum0=4224, sel=4352, red=4864, wd=4873, ybias=4891,
    pq=4892, total=5020,
)
# fp16 constant tile columns ([2, *]): dl lhsT per seg
CXTRA_COLS = dict(dl=0, total=256)


def _sig_rho(ul, h2, g):
    return 32 * (ul & 3) + 8 * (ul >> 2) + 4 * h2 + g


def _rgb_rho(ul, g, c2):
    return 32 * ((ul + 2) & 3) + 6 * g + c2


def build_constants(W1, b1, Wsig, Wsig_d, Wc1, bc1, Wc2, Wc2_d):
    C = {}
    lhsT_H = np.zeros((32, 64, 128), F32)
    for u in range(32):
        for g in range(4):
            for j in range(H):
                lhsT_H[u, j, 32 * g + j] = 1.0
                lhsT_H[u, H + j, 32 * g + j] = Z[4 * u + g]
    C["lhsT_H"] = lhsT_H

    Wsig2 = [np.asarray(Wsig, F32)[:, 0], np.asarray(Wsig_d, F32)[:, 0]]
    lhsT_sig = np.zeros((4, 128, 32), F32)
    for qp in range(4):
        for g in range(4):
            for h2 in range(2):
                for j in range(H):
                    lhsT_sig[qp, 32 * g + j, 8 * qp + 4 * h2 + g] = Wsig2[h2][j]
    C["lhsT_sig"] = lhsT_sig

    Wc2all = np.concatenate([np.asarray(Wc2, F32), np.asarray(Wc2_d, F32)], axis=1)
    lhsT_rgb = np.zeros((128, 32), F32)
    for g in range(4):
        for c2 in range(6):
            for j in range(H):
                lhsT_rgb[32 * g + j, 6 * g + c2] = Wc2all[j, c2]
    C["lhsT_rgb"] = lhsT_rgb

    rho_t = np.zeros(128, np.int64)
    rho_h2 = np.zeros(128, np.int64)
    for ul in range(16):
        for h2 in range(2):
            for g in range(4):
                rho = _sig_rho(ul, h2, g)
                rho_t[rho] = 4 * ul + g
                rho_h2[rho] = h2
    C["lhsT_tri"] = ((rho_h2[:, None] == rho_h2[None, :])
                     & (rho_t[:, None] <= rho_t[None, :])).astype(F32)
    C["lhsT_sum0"] = (rho_h2[:, None] == rho_h2[None, :]).astype(F32)

    lhsT_sel = np.zeros((4, 128, 128), F32)
    for beta in range(4):
        for ul in range(4 * beta, 4 * beta + 4):
            for g in range(4):
                for c2 in range(6):
                    rr = _rgb_rho(ul, g, c2)
                    h2 = 1 if c2 >= 3 else 0
                    src = np.where((rho_t == 4 * ul + g) & (rho_h2 == h2))[0]
                    lhsT_sel[beta, src[0], rr] = 1.0
    C["lhsT_sel"] = lhsT_sel

    lhsT_red = np.zeros((128, 6), F32)
    for rr in range(128):
        c24 = rr & 31
        if c24 < 24:
            lhsT_red[rr, c24 % 6] = 1.0
    C["lhsT_red"] = lhsT_red

    lhsT_wd = np.zeros((2, 128, 4), F32)
    for seg in range(2):
        for rho in range(128):
            h2 = rho_h2[rho]
            lhsT_wd[seg, rho, 2 * h2 + 0] = 1.0
            lhsT_wd[seg, rho, 2 * h2 + 1] = Z[64 * seg + rho_t[rho]]
    C["lhsT_wd"] = lhsT_wd

    # y assembly folded into the reductions.  rgb is shipped as tanh(raw/2);
    # sigmoid = 0.5*tanh + 0.5 folded here:
    # image_final = 0.5*imgth + 0.5*ws + (1 - ws) = 0.5*imgth - 0.5*ws + 1
    # with the "+1" applied as a per-partition bias on the output copy.
    # y rows: image(3) | depth(1) | ws(1) | image_d(3) | depth_d(1)
    ylhs_img = np.zeros((6, 9), F32)
    for c2 in range(6):
        ylhs_img[c2, c2 if c2 < 3 else 2 + c2] = 0.5
    ylhs_wd = np.zeros((4, 9), F32)
    ylhs_wd[0, 0:3] = -0.5
    ylhs_wd[0, 4] = 1.0
    ylhs_wd[1, 3] = 1.0
    ylhs_wd[2, 5:8] = -0.5
    ylhs_wd[3, 8] = 1.0
    C["lhsT_red_y"] = C["lhsT_red"] @ ylhs_img          # [128, 9]
    C["lhsT_wd_y"] = np.stack([C["lhsT_wd"][s_] @ ylhs_wd for s_ in range(2)])
    C["ybias"] = np.array([1, 1, 1, 0, 0, 1, 1, 1, 0], F32)

    lhsT_dl = np.zeros((2, 2, 128), F32)
    for seg in range(2):
        for rho in range(128):
            tg = 64 * seg + rho_t[rho]
            lhsT_dl[seg, 0 if tg != 127 else 1, rho] = 1.0
    C["lhsT_dl"] = lhsT_dl

    # pq lhsT [10, 128] applied directly to the fp16 ray rows
    # (A3, B3, d3, ones): columns 0-31 P, 32-63 Q, 64-95 Pc, 96-127 Qc
    W1 = np.asarray(W1, F32)
    b1 = np.asarray(b1, F32)
    Wc1 = np.asarray(Wc1, F32)
    bc1 = np.asarray(bc1, F32)
    pq = np.zeros((10, 128), F32)
    for j in range(H):
        for ci in range(3):
            pq[ci, j] = W1[ci, j]            # P
            pq[3 + ci, 32 + j] = W1[ci, j]   # Q
            pq[ci, 64 + j] = Wc1[ci, j]      # Pc (A part)
            pq[6 + ci, 64 + j] = Wc1[3 + ci, j]  # Pc (d part)
            pq[3 + ci, 96 + j] = Wc1[ci, j]  # Qc
        pq[9, j] = b1[j]
        pq[9, 64 + j] = bc1[j]
    C["lhsT_pq"] = pq
    return C


def pack_const_tiles(C):
    CC = CONST_COLS
    ct = np.zeros((128, CC["total"]), F32)
    for u in range(32):
        ct[0:64, 128 * u:128 * u + 128] = C["lhsT_H"][u]
        ct[64:128, 128 * u:128 * u + 128] = C["lhsT_H"][u]
    ct[:, CC["tri"]:CC["tri"] + 128] = C["lhsT_tri"]
    ct[:, CC["sum0"]:CC["sum0"] + 128] = C["lhsT_sum0"]
    for b in range(4):
        ct[:, CC["sel"] + 128 * b:CC["sel"] + 128 * b + 128] = C["lhsT_sel"][b]
    ct[:, CC["red"]:CC["red"] + 9] = C["lhsT_red_y"]
    for seg in range(2):
        ct[:, CC["wd"] + 9 * seg:CC["wd"] + 9 * seg + 9] = C["lhsT_wd_y"][seg]
    ct[0:9, CC["ybias"]] = C["ybias"]

    ct[0:10, CC["pq"]:CC["pq"] + 128] = C["lhsT_pq"]

    CX = CXTRA_COLS
    cx = np.zeros((2, CX["total"]), F32)
    for seg in range(2):
        cx[0:2, CX["dl"] + 128 * seg:CX["dl"] + 128 * seg + 128] = C["lhsT_dl"][seg]

    c16 = np.zeros((128, 160), F32)
    for qp in range(4):
        c16[:, 32 * qp:32 * qp + 32] = C["lhsT_sig"][qp]
    c16[:, 128:160] = C["lhsT_rgb"]
    return ct, cx.astype(np.float16), c16


def host_prep(rays_o, rays_d):
    """Per-ray prep -> [12, N] rows (A3, B3, d3, ones, dl0, dl1)."""
    o = np.asarray(rays_o, F32)
    rd = np.asarray(rays_d, F32)
    n2 = rd[:, 0] * rd[:, 0] + rd[:, 1] * rd[:, 1] + rd[:, 2] * rd[:, 2]
    d = rd * (1.0 / np.sqrt(n2))[:, None]
    inv = 1.0 / d
    t1 = (-1.0 - o) * inv
    t2 = (1.0 - o) * inv
    near = np.maximum(np.minimum(t1, t2).max(-1), F32(0.2))
    far = np.maximum(np.maximum(t1, t2).min(-1), near + F32(1e-6))
    span = far - near
    A = o + d * near[:, None]
    B = d * span[:, None]
    N = o.shape[0]
    R12 = np.empty((12, N), F32)
    R12[0:3] = A.T
    R12[3:6] = B.T
    R12[6:9] = d.T
    R12[9] = 1.0
    R12[10] = span * (1.0 / (T - 1))
    R12[11] = span * (1.0 / T)
    return R12


def emit_nerf(tc, y_ap, x_ap, cst_ap, cx_ap, c16_ap, n_rays=R_CORE):
    import concourse.mybir as mybir
    AF = mybir.ActivationFunctionType
    ALU = mybir.AluOpType
    f32 = mybir.dt.float32
    f32r = mybir.dt.float32r
    bf16 = mybir.dt.bfloat16
    f16 = mybir.dt.float16
    nc = tc.nc
    nchunk = n_rays // RC
    CC = CONST_COLS
    CX = CXTRA_COLS

    with ExitStack() as ctx:
        singles = ctx.enter_context(tc.tile_pool(name="singles", bufs=1))
        xpool = ctx.enter_context(tc.tile_pool(name="xpool", bufs=2))
        hpool = ctx.enter_context(tc.tile_pool(name="hpool", bufs=38))
        cpool = ctx.enter_context(tc.tile_pool(name="cpool", bufs=2))
        rgbpool = ctx.enter_context(tc.tile_pool(name="rgbpool", bufs=8))
        opool = ctx.enter_context(tc.tile_pool(name="opool", bufs=2))
        psBig = ctx.enter_context(tc.tile_pool(name="psBig", bufs=2, space="PSUM"))
        psSig = ctx.enter_context(tc.tile_pool(name="psSig", bufs=2, space="PSUM"))
        psAux = ctx.enter_context(tc.tile_pool(name="psAux", bufs=2, space="PSUM"))

        cx16 = singles.tile([2, CX["total"]], f16)
        nc.sync.dma_start(out=cx16[:], in_=cx_ap[:])
        c16 = singles.tile([128, 160], bf16)
        nc.sync.dma_start(out=c16[:], in_=c16_ap[:])
        cst = singles.tile([128, CC["total"]], f32r)
        nc.sync.dma_start(out=cst[:], in_=cst_ap[:])

        def cs(key, off, k, w):
            c0 = CC[key] + off
            return cst[0:k, c0:c0 + w] if k != 128 else cst[:, c0:c0 + w]

        def MM(out, lhsT, rhs, **kw):
            # float32r: same bytes, 4x faster PE row rate at N>=256
            nc.tensor.matmul(out, lhsT.bitcast(f32r), rhs.bitcast(f32r), **kw)

        def phase_b(c, st):
            """Hidden evals + relu evacs + sigma head matmuls.

            Generator: yields once per t-quad (32 times) so the driver can
            interleave the previous chunk's composite between quads.
            """
            r16 = xpool.tile([10, RC], f16, tag="r16", name=f"r16{c}")
            nc.sync.dma_start(out=r16[:], in_=x_ap[0:10, c * RC:(c + 1) * RC])
            d16 = xpool.tile([2, RC], f16, tag="d16", name=f"d16{c}")
            nc.sync.dma_start(out=d16[:], in_=x_ap[10:12, c * RC:(c + 1) * RC])
            st["d16"] = d16
            r_c = xpool.tile([10, RC], f32r, tag="rc", name=f"rc{c}")
            nc.vector.tensor_copy(r_c[:], r16[:])
            x_ps = psSig.tile([128, RC], f32, tag="sig", name=f"xps{c}")
            MM(x_ps[:], cs("pq", 0, 10, 128), r_c[:], start=True, stop=True)
            x_c = xpool.tile([128, RC], f32r, tag="xc", name=f"xc{c}")
            nc.scalar.activation(x_c[:], x_ps[:], AF.Copy)

            sig_ps = [None, None]
            st["a"] = [None, None]
            st["husbs"] = husbs = []

            def sig_head(u, husb):
                ul = u & 15
                s = ul & 3
                qp = ul >> 2
                seg = u >> 4
                nc.tensor.matmul(
                    sig_ps[seg][32 * s:32 * s + 32, :],
                    c16[:, 32 * qp:32 * qp + 32], husb[:, 0:RC],
                    start=(qp == 0), stop=(qp == 3),
                    tile_position=(0, 32 * s), skip_group_check=True)

            pend = []  # (u, husb) lagged 3 quads so evacs stay ahead of PE
            for u in range(32):
                seg = u >> 4
                if (u & 15) == 0:
                    sig_ps[seg] = psSig.tile([128, RC], f32, tag="sig",
                                             name=f"sig{c}_{seg}")
                hbig = psBig.tile([128, 2 * RC], f32, tag="big", name=f"hb{c}_{u}")
                MM(hbig[:, 0:RC], cst[0:64, 128 * u:128 * (u + 1)], x_c[0:64, :],
                   start=True, stop=True)
                MM(hbig[:, RC:2 * RC], cst[64:128, 128 * u:128 * (u + 1)],
                   x_c[64:128, :], start=True, stop=True)
                if len(pend) >= 3:
                    sig_head(*pend.pop(0))
                husb = hpool.tile([128, 2 * RC], bf16, tag="husb", bufs=38,
                                  name=f"hu{c}_{u}")
                # Bresenham split of the 32 relu evacs: ~15 on ScalarE,
                # ~17 on VectorE (balances total per-engine busy time)
                if (u * 18) // 32 != ((u + 1) * 18) // 32:
                    nc.scalar.activation(husb[:], hbig[:], AF.Relu)
                else:
                    nc.vector.tensor_scalar_max(husb[:], hbig[:], 0.0)
                husbs.append(husb)
                pend.append((u, husb))
                if (u & 15) == 15:
                    for p in pend:
                        sig_head(*p)
                    pend = []
                    # sigma pre-activations complete for this seg
                    a_sb = cpool.tile([128, RC], f32, tag="a", name=f"a{c}_{seg}")
                    nc.scalar.activation(a_sb[:], sig_ps[seg][:], AF.Exp)
                    st["a"][seg] = a_sb
                yield

        def composite(c, st):
            """B2 (rgb heads + tanh) then C1 (softplus/cumsum/weights) then
            C2 (replication + weighted reduction).  Generator yielding at op
            boundaries; interleaved into the NEXT chunk's phase B."""
            d16 = st["d16"]
            husbs = st["husbs"]
            a_sbs = st["a"]

            # B2: rgb head matmuls + tanh evacs (frees husb slots early)
            rgb_sb = [[None] * 4, [None] * 4]
            for seg in range(2):
                for beta in range(4):
                    rps = psAux.tile([128, RC], f32, tag="aux",
                                     name=f"rgbps{c}_{seg}_{beta}")
                    for ul in range(4 * beta, 4 * beta + 4):
                        sr = (ul + 2) & 3
                        nc.tensor.matmul(
                            rps[32 * sr:32 * sr + 32, :],
                            c16[:, 128:160], husbs[16 * seg + ul][:, RC:2 * RC],
                            start=True, stop=True,
                            tile_position=(0, 32 * sr), skip_group_check=True)
                        if ul & 1:
                            yield
                    r_sb = rgbpool.tile([128, RC], f32, tag="rgbsb",
                                        name=f"rgbsb{c}_{seg}_{beta}")
                    nc.scalar.activation(r_sb[:], rps[:], AF.Tanh, scale=0.5)
                    rgb_sb[seg][beta] = r_sb
            st["husbs"] = []
            # tiny column derived from the last tanh output: used as the Ln
            # bias (=1.0) so Ln cannot be scheduled before the tanh group
            # (keeps one table-set switch per group per chunk)
            dep = cpool.tile([128, 1], f32, tag="dep", name=f"dep{c}")
            nc.vector.tensor_scalar(dep[:], rgb_sb[1][3][:, 0:1], 0.0, 1.0,
                                    op0=ALU.mult, op1=ALU.add)
            yield

            # C1: softplus (exp+ln), deltas, cumsum, weights
            dl_ps = [None, None]
            for seg in range(2):
                dl = psAux.tile([128, RC], f32, tag="aux", name=f"dl{c}_{seg}")
                nc.tensor.matmul(dl[:], cx16[0:2, CX["dl"] + 128 * seg:
                                              CX["dl"] + 128 * (seg + 1)],
                                 d16[:], start=True, stop=True)
                dl_ps[seg] = dl
            yield
            L_sbs = []
            for seg in range(2):
                L_sb = cpool.tile([128, RC], f32, tag="L", name=f"L{c}_{seg}")
                nc.scalar.activation(L_sb[:], a_sbs[seg][:], AF.Ln, bias=dep[:])
                L_sbs.append(L_sb)
            yield
            x_sb = [None, None]
            for seg in range(2):
                xs = cpool.tile([128, RC], f32r, tag="x", name=f"x{c}_{seg}")
                nc.vector.tensor_tensor(xs[:], L_sbs[seg][:], dl_ps[seg][:],
                                        op=ALU.mult)
                x_sb[seg] = xs
            yield
            w_sb = [None, None]
            for seg in range(2):
                S_ps = psAux.tile([128, RC], f32, tag="aux", name=f"S{c}_{seg}")
                MM(S_ps[:], cs("tri", 0, 128, 128), x_sb[seg][:],
                   start=True, stop=(seg == 0))
                if seg == 1:
                    MM(S_ps[:], cs("sum0", 0, 128, 128), x_sb[0][:],
                       start=False, stop=True)
                tmp = cpool.tile([128, RC], f32, tag="tmp", name=f"tmp{c}_{seg}")
                nc.vector.tensor_tensor(tmp[:], x_sb[seg][:], S_ps[:],
                                        op=ALU.subtract)
                E_sb = cpool.tile([128, RC], f32r, tag="E", name=f"E{c}_{seg}")
                nc.scalar.activation(E_sb[:], tmp[:], AF.Exp)
                F_sb = cpool.tile([128, RC], f32r, tag="F", name=f"F{c}_{seg}")
                nc.scalar.activation(F_sb[:], S_ps[:], AF.Exp, scale=-1.0)
                ws_ = cpool.tile([128, RC], f32r, tag="w", name=f"w{c}_{seg}")
                nc.vector.tensor_tensor(ws_[:], E_sb[:], F_sb[:],
                                        op=ALU.subtract)
                w_sb[seg] = ws_
                yield

            # C2: weight replication, weighted reduction into y
            y_ps = psAux.tile([9, RC], f32, tag="aux", name=f"y{c}")
            MM(y_ps[:], cs("wd", 0, 128, 9), w_sb[0][:],
               start=True, stop=False, skip_group_check=True)
            MM(y_ps[:], cs("wd", 9, 128, 9), w_sb[1][:],
               start=False, stop=False, skip_group_check=True)
            yield
            n_img = 0
            for seg in range(2):
                for beta in range(4):
                    wrep_ps = psAux.tile([128, RC], f32, tag="aux",
                                         name=f"wrep{c}_{seg}_{beta}")
                    MM(wrep_ps[:], cs("sel", 128 * beta, 128, 128),
                       w_sb[seg][:], start=True, stop=True)
                    wrgb = cpool.tile([128, RC], f32r, tag="wrgb",
                                      name=f"wrgb{c}_{seg}_{beta}")
                    nc.vector.tensor_tensor(wrgb[:], rgb_sb[seg][beta][:],
                                            wrep_ps[:], op=ALU.mult)
                    MM(y_ps[:], cs("red", 0, 128, 9), wrgb[:],
                       start=False, stop=(n_img == 7),
                       skip_group_check=True)
                    n_img += 1
                    yield

            y_sb = opool.tile([9, RC], f16, tag="ysb", name=f"ysb{c}")
            nc.scalar.activation(y_sb[:], y_ps[:], AF.Identity,
                                 bias=cs("ybias", 0, 9, 1))
            nc.sync.dma_start(out=y_ap[:, c * RC:(c + 1) * RC], in_=y_sb[:])

        # software pipeline: interleave composite(c-1) into phase_b(c)
        states = {}
        prev_comp = None
        for c in range(nchunk):
            states[c] = {}
            bgen = phase_b(c, states[c])
            for _ in bgen:
                if prev_comp is not None:
                    next(prev_comp, None)
            if prev_comp is not None:
                for _ in prev_comp:
                    pass
                del states[c - 1]
            prev_comp = composite(c, states[c])
        for _ in prev_comp:
            pass


_CACHED = {}


def _build_runner(nc):
    """Persistent jitted SPMD runner (avoids bass2jax's per-call re-jit)."""
    import jax
    from jax.sharding import Mesh, PartitionSpec
    from jax.experimental.shard_map import shard_map
    from concourse import bass2jax

    bass2jax.install_neuronx_cc_hook()
    in_names = ["x", "cst", "cxtra", "cst16"]
    out_names = ["y"]
    out_avals = [jax.core.ShapedArray((9, R_CORE), np.float16)]
    all_names = in_names + out_names
    pname = nc.partition_id_tensor.name if nc.partition_id_tensor else None
    if pname is not None:
        all_names = all_names + [pname]

    def _body(*args):
        operands = list(args)
        if pname is not None:
            operands.append(bass2jax.partition_id_tensor())
        outs = bass2jax._bass_exec_p.bind(
            *operands,
            out_avals=tuple(out_avals),
            in_names=tuple(all_names),
            out_names=tuple(out_names),
            lowering_input_output_aliases=(),
            sim_require_finite=True,
            sim_require_nnan=True,
            nc=nc,
        )
        return tuple(outs)

    import jax.numpy as jnp
    from jax.sharding import NamedSharding

    devices = jax.devices()[:N_CORES]
    mesh = Mesh(np.asarray(devices), ("core",))
    sh = NamedSharding(mesh, PartitionSpec("core"))

    sharded = jax.jit(
        shard_map(_body, mesh=mesh,
                  in_specs=(PartitionSpec("core"),) * 5,
                  out_specs=(PartitionSpec("core"),) * len(out_names),
                  check_rep=False),
        keep_unused=True)
    zfn = jax.jit(lambda: jnp.zeros((N_CORES * 9, R_CORE), np.float16),
                  out_shardings=sh)
    cst_cache = {}

    def run(x_cat, cst, cx, c16, cst_key):
        if cst_key not in cst_cache:
            cst_cache.clear()
            cst_cache[cst_key] = (
                jax.device_put(np.concatenate([cst] * N_CORES, axis=0), sh),
                jax.device_put(np.concatenate([cx] * N_CORES, axis=0), sh),
                jax.device_put(np.concatenate([c16] * N_CORES, axis=0), sh))
        # z is created fresh per call: besides providing the output-alias
        # operand, executing the (different) zeros NEFF between kernel
        # executions resets device state -- back-to-back executions of the
        # same NEFF otherwise see stale state and produce wrong results.
        z = zfn()
        cd, cxd, c16d = cst_cache[cst_key]
        (y_out,) = sharded(x_cat, cd, cxd, c16d, z)
        out = np.asarray(y_out).reshape(N_CORES, 9, R_CORE)
        return out

    return run


def _build_module():
    import concourse.bacc as bacc
    import concourse.tile as tile
    import concourse.mybir as mybir

    nc = bacc.Bacc("TRN2", target_bir_lowering=False, debug=False)
    x = nc.dram_tensor("x", [12, R_CORE], mybir.dt.float16, kind="ExternalInput")
    cst = nc.dram_tensor("cst", [128, CONST_COLS["total"]], mybir.dt.float32r,
                         kind="ExternalInput")
    cx = nc.dram_tensor("cxtra", [2, CXTRA_COLS["total"]], mybir.dt.float16,
                        kind="ExternalInput")
    c16 = nc.dram_tensor("cst16", [128, 160], mybir.dt.bfloat16,
                         kind="ExternalInput")
    y = nc.dram_tensor("y", [9, R_CORE], mybir.dt.float16, kind="ExternalOutput")
    with tile.TileContext(nc) as tc:
        emit_nerf(tc, y.ap(), x.ap(), cst.ap(), cx.ap(), c16.ap(), n_rays=R_CORE)
    nc.compile()
    return nc


def _prep_consts(weights):
    import hashlib
    key = hashlib.md5(b"".join(a.tobytes() for a in weights)).hexdigest()
    if _CACHED.get("cst_key") != key:
        C = build_constants(*weights)
        ct, cx, c16 = pack_const_tiles(C)
        _CACHED["cst"] = ct
        _CACHED["cxtra"] = cx
        try:
            import ml_dtypes
            _CACHED["cst16"] = c16.astype(ml_dtypes.bfloat16)
        except ImportError:
            _CACHED["cst16"] = c16.astype(np.dtype("bfloat16"))
        _CACHED["cst_key"] = key
    return key


def kernel(rays_o, rays_d, W1, b1, Wsig, Wsig_d, Wc1, bc1, Wc2, Wc2_d, num_steps):
    assert int(num_steps) == T
    weights = [np.ascontiguousarray(np.asarray(a, F32))
               for a in (W1, b1, Wsig, Wsig_d, Wc1, bc1, Wc2, Wc2_d)]
    key = _prep_consts(weights)

    R12 = host_prep(rays_o, rays_d).astype(np.float16)
    # concat over cores: [N_CORES*12, R_CORE] (shard_map splits on axis 0)
    x_cat = np.ascontiguousarray(
        R12.reshape(12, N_CORES, R_CORE).transpose(1, 0, 2).reshape(
            N_CORES * 12, R_CORE))

    if "run" not in _CACHED:
        _CACHED["nc"] = _build_module()
        _CACHED["run"] = _build_runner(_CACHED["nc"])

    y = _CACHED["run"](x_cat, _CACHED["cst"], _CACHED["cxtra"],
                       _CACHED["cst16"], key)  # [N_CORES, 9, R_CORE] fp16
    out = np.concatenate([y[cidx].T for cidx in range(N_CORES)], axis=0)
    return np.ascontiguousarray(out.astype(np.float32))


if __name__ == "__main__":
    rng = np.random.default_rng(0)
    ins = {
        "rays_o": (rng.random((N_RAYS, 3), dtype=np.float32) - 0.5),
        "rays_d": rng.standard_normal((N_RAYS, 3)).astype(np.float32),
        "W1": rng.standard_normal((3, 32)).astype(np.float32) * 0.5,
        "b1": np.zeros((32,), np.float32),
        "Wsig": rng.standard_normal((32, 1)).astype(np.float32) * 0.5,
        "Wsig_d": rng.standard_normal((32, 1)).astype(np.float32) * 0.5,
        "Wc1": rng.standard_normal((6, 32)).astype(np.float32) * 0.5,
        "bc1": np.zeros((32,), np.float32),
        "Wc2": rng.standard_normal((32, 3)).astype(np.float32) * 0.5,
        "Wc2_d": rng.standard_normal((32, 3)).astype(np.float32) * 0.5,
        "num_steps": 128,
    }
    out = kernel(**ins)
    print("out", out.shape, out.dtype, np.isfinite(out).all())
